# revision 17
# baseline (speedup 1.0000x reference)
"""ModAFNO2D layer as a Bass/Tile kernel for 8 Trainium2 NeuronCores.

Sharding: 8 cores = (batch b in 0..3) x (block-half in 0..1). Each core owns one
batch sample and 4 of the 8 FNO blocks (= 384 of 768 channels). The FFT axes are
per-channel and channel blocks never mix, so cores are fully independent — no
collectives; host slices inputs and concatenates outputs.

Per-core pipeline (all heavy math on the PE as bf16 matmuls; DFTs as matrix
multiplies with precomputed 128-point DFT matrices):
  A : Z^T = X_c^T @ [Fr|Fi]                 (FFT along H; X_c stationary)
  B : Y[c,(Yr|Yi)] at fixed h'              (rFFT along W)
  mix: block-diagonal 2-layer complex MLP with adaLN modulation. The second
       layer's imag output is rewritten as i2 = A2@r1 + B2@i1 + c2 with
       A2 = w2r@w2i, B2 = w2r - w2i@w2i, c2 = b2r@w2i + b2i (computed once on
       device), which removes the r2->i2 serial dependency.
  T : PE transposes [c,wf]->[wf,c] to pivot back to spatial-major
  E': [Pr|Pi] = Z @ [Sr|Si]                 (inverse rFFT along W)
  D': out = FHr@Pr - FHi@Pi + x             (inverse FFT along H + residual)
All spectra/activations bf16 (PSUM accumulation fp32); input x is shipped
pre-transposed [h, c, w] in bf16 and the residual/output stay bf16 (host
casts back to fp32). The residual add rides the last D' matmul (identity
stationary). PSUM evictions are spread over DVE/ACT; the SBUF-only softshrink
clip/sub ops run on GpSimd (Pool); all DMAs issue from the otherwise-idle SP
queue. Block-0's X prefetch and stage A are hoisted ahead of the one-time
setup (modulation, W2 combos) so the PE starts ~5us into the program.
"""

import numpy as np
import ml_dtypes

BF16 = ml_dtypes.bfloat16
F8 = ml_dtypes.float8_e4m3

DIM = 768
NB = 8
BS = 96
LAM = 0.01
B_FULL = 4
H = 128
W = 128
WF = W // 2 + 1  # 65
NBL = 4          # blocks per core
C = NBL * BS     # 384 channels per core
N_CORES = 8
HC = 4           # h' rows per fused B/mix/T chunk
CSUB = 24        # channels per E/D sub-group (Pbuf ring tile)


def _host_consts():
    jh = np.arange(H)
    F = np.exp(-2j * np.pi * np.outer(jh, jh) / H)
    R = np.exp(-2j * np.pi * np.outer(np.arange(WF), np.arange(W)) / W) / 128.0
    RrT, RiT = R.real.T, R.imag.T                      # [w, wf]
    FH = np.conj(F)
    cw = np.ones(WF)
    cw[1:-1] = 2.0
    S = (cw[:, None] * np.exp(2j * np.pi * np.outer(np.arange(WF), np.arange(W)) / W)) / 128.0
    consts = {
        "cF": np.concatenate([F.real, F.imag], 1).astype(BF16),            # [128, 256]
        "cBp": (16.0 * np.stack(
            [np.concatenate([RrT, RiT], 1),
             np.concatenate([-RiT, RrT], 1)], axis=1)).astype(F8),         # [128, 2, 130]
        "cE1": np.concatenate([S.real, S.imag], 1).astype(BF16),           # [65, 256]
        "cE2": np.concatenate([-S.imag, S.real], 1).astype(BF16),          # [65, 256]
        "cDr": FH.real.astype(BF16),                                       # [128, 128]
        "cDi": (-FH.imag).astype(BF16),                                    # [128, 128]
        "cI": np.eye(128, dtype=np.float32).astype(BF16),                  # [128, 128]
    }
    return consts


def _build_program():
    from contextlib import ExitStack

    import concourse.bass as bass  # noqa: F401
    import concourse.mybir as mybir
    import concourse.tile as tile
    from concourse import bacc

    f32 = mybir.dt.float32
    bf = mybir.dt.bfloat16
    f8 = mybir.dt.float8e4
    DRow = mybir.MatmulPerfMode.DoubleRow
    AF = mybir.ActivationFunctionType
    ALU = mybir.AluOpType

    nc = bacc.Bacc("TRN2", target_bir_lowering=False, debug=False)

    xhw = nc.dram_tensor("xhw", [H, C, W], bf, kind="ExternalInput")
    w1pr = nc.dram_tensor("w1pr", [BS, 2, NBL, BS], f8, kind="ExternalInput")
    w1pi = nc.dram_tensor("w1pi", [BS, 2, NBL, BS], f8, kind="ExternalInput")
    w2r = nc.dram_tensor("w2r", [BS, NBL, BS], bf, kind="ExternalInput")
    w2i = nc.dram_tensor("w2i", [BS, NBL, BS], bf, kind="ExternalInput")
    nw2i = nc.dram_tensor("nw2i", [BS, NBL, BS], bf, kind="ExternalInput")
    w2Ad = nc.dram_tensor("w2Ad", [BS, NBL, BS], bf, kind="ExternalInput")
    w2Bd = nc.dram_tensor("w2Bd", [BS, NBL, BS], bf, kind="ExternalInput")
    c2d = nc.dram_tensor("c2d", [BS, NBL], f32, kind="ExternalInput")
    shp1d = nc.dram_tensor("shp1d", [BS, NBL], f32, kind="ExternalInput")
    shp1sd = nc.dram_tensor("shp1sd", [BS, NBL], f32, kind="ExternalInput")
    addrd = nc.dram_tensor("addrd", [BS, NBL], f32, kind="ExternalInput")
    addid = nc.dram_tensor("addid", [BS, NBL], f32, kind="ExternalInput")
    b2rd = nc.dram_tensor("b2rd", [BS, NBL], f32, kind="ExternalInput")
    b2id = nc.dram_tensor("b2id", [BS, NBL], f32, kind="ExternalInput")
    cF = nc.dram_tensor("cF", [H, 2 * H], bf, kind="ExternalInput")
    cBp = nc.dram_tensor("cBp", [W, 2, 2 * WF], f8, kind="ExternalInput")
    cE1 = nc.dram_tensor("cE1", [WF, 2 * W], bf, kind="ExternalInput")
    cE2 = nc.dram_tensor("cE2", [WF, 2 * W], bf, kind="ExternalInput")
    cDr = nc.dram_tensor("cDr", [H, H], bf, kind="ExternalInput")
    cDi = nc.dram_tensor("cDi", [H, H], bf, kind="ExternalInput")
    cI = nc.dram_tensor("cI", [128, 128], bf, kind="ExternalInput")
    outs = nc.dram_tensor("outs", [H, C, W], bf, kind="ExternalOutput")

    with ExitStack() as ctx:
        tc = ctx.enter_context(tile.TileContext(nc))
        consts = ctx.enter_context(tc.tile_pool(name="consts", bufs=1))
        xpool = ctx.enter_context(tc.tile_pool(name="xpool", bufs=2))
        zpool = ctx.enter_context(tc.tile_pool(name="zpool", bufs=1))
        planep = ctx.enter_context(tc.tile_pool(name="planep", bufs=1))
        pbufp = ctx.enter_context(tc.tile_pool(name="pbufp", bufs=3))
        mixp = ctx.enter_context(tc.tile_pool(name="mixp", bufs=2))
        outp = ctx.enter_context(tc.tile_pool(name="outp", bufs=6))
        psum = ctx.enter_context(tc.tile_pool(name="psum", bufs=2, space="PSUM"))

        # ---- stage-A DFT matrix first, then block-0 X prefetch (SP queue) ----
        cF_sb = consts.tile([H, 2 * H], bf)
        nc.sync.dma_start(cF_sb, cF[:])
        X16_first = xpool.tile([H, BS, W], bf, tag="xblk")
        for cc in range(0, BS, CSUB):
            nc.sync.dma_start(
                X16_first[:, cc: cc + CSUB, :], xhw[:, cc: cc + CSUB, :]
            )
        # ---- modulation vectors computed on host: shp1=shift+1, shp1_s=shp1/128,
        # addv = b1*shp1 + scale (ships 4 tiny f32 vectors instead of mwT) ----
        shp1 = consts.tile([BS, NBL], f32)
        shp1_s = consts.tile([BS, NBL], f32)
        addr_v = consts.tile([BS, NBL], f32)
        addi_v = consts.tile([BS, NBL], f32)
        nc.sync.dma_start(shp1, shp1d[:])
        nc.sync.dma_start(shp1_s, shp1sd[:])
        nc.sync.dma_start(addr_v, addrd[:])
        nc.sync.dma_start(addi_v, addid[:])

        cBp_sb = consts.tile([W, 2, 2 * WF], f8)
        nc.sync.dma_start(cBp_sb, cBp[:])
        cE1_sb = consts.tile([WF, 2 * W], bf)
        nc.sync.dma_start(cE1_sb, cE1[:])
        cE2_sb = consts.tile([WF, 2 * W], bf)
        nc.sync.dma_start(cE2_sb, cE2[:])
        cDr_sb = consts.tile([H, H], bf)
        nc.sync.dma_start(cDr_sb, cDr[:])
        cDi_sb = consts.tile([H, H], bf)
        nc.sync.dma_start(cDi_sb, cDi[:])
        cI_sb = consts.tile([128, 128], bf)
        nc.sync.dma_start(cI_sb, cI[:])

        # ---- block weights ----
        w1pr_sb = consts.tile([BS, 2, NBL, BS], f8)
        w1pi_sb = consts.tile([BS, 2, NBL, BS], f8)
        w2r_sb = consts.tile([BS, NBL, BS], bf)
        w2i_sb = consts.tile([BS, NBL, BS], bf)
        nw2i_sb = consts.tile([BS, NBL, BS], bf)
        w2A_sb = consts.tile([BS, NBL, BS], bf)   # w2r @ w2i (host)
        w2B_sb = consts.tile([BS, NBL, BS], bf)   # w2r - w2i @ w2i (host)
        for t_sb_, t_dr_ in ((w1pr_sb, w1pr), (w1pi_sb, w1pi),
                             (w2r_sb, w2r), (w2i_sb, w2i), (nw2i_sb, nw2i),
                             (w2A_sb, w2Ad), (w2B_sb, w2Bd)):
            nc.sync.dma_start(t_sb_, t_dr_[:])
        b2r_v = consts.tile([BS, NBL], f32)
        c2_v = consts.tile([BS, NBL], f32)        # b2r @ w2i + b2i (host)
        nc.sync.dma_start(b2r_v, b2rd[:])
        nc.sync.dma_start(c2_v, c2d[:])

        def stage_a(X16, Zbuf, deep=False):
            for cp in range(BS // 2):
                c = 2 * cp
                if deep and cp % 2 == 1:
                    # startup only: other rings are idle, deepen the pipeline
                    pA = psum.tile([128, 2, 2 * H], f32, tag="ps_m", bufs=3)
                else:
                    pA = psum.tile([128, 2, 2 * H], f32, tag="ps_b")
                nc.tensor.matmul(pA[:, 0, :], lhsT=X16[:, c, :], rhs=cF_sb,
                                 start=True, stop=True)
                nc.tensor.matmul(pA[:, 1, :], lhsT=X16[:, c + 1, :], rhs=cF_sb,
                                 start=True, stop=True)
                pAr = pA.rearrange("w c (r h) -> w r c h", r=2)
                if cp % 2 == 0:
                    nc.vector.tensor_copy(Zbuf[:, :, c: c + 2, :], pAr)
                else:
                    nc.scalar.copy(Zbuf[:, :, c: c + 2, :], pAr)

        # stage A of block 0 runs before the one-time setup sections so the
        # PE starts as soon as the first X chunk lands
        Zbuf_first = zpool.tile([W, 2, BS, H], f8, tag="zbuf")
        stage_a(X16_first, Zbuf_first, deep=True)



        # ---- main per-block pipeline ----
        for n in range(NBL):
            c0 = n * BS

            # resident X for this block: [h, c, w] bf16 (stage-A lhsT + residual)
            if n == 0:
                X16 = X16_first
            else:
                X16 = xpool.tile([H, BS, W], bf, tag="xblk")
                for cc in range(0, BS, CSUB):
                    nc.sync.dma_start(
                        X16[:, cc: cc + CSUB, :],
                        xhw[:, c0 + cc: c0 + cc + CSUB, :],
                    )

            # ---- stage A: Z^T = X_c^T @ [Fr|Fi] -> Zbuf [w, c, h'Zr|h'Zi] ----
            if n == 0:
                Zbuf = Zbuf_first
            else:
                Zbuf = zpool.tile([W, 2, BS, H], f8, tag="zbuf")
                stage_a(X16, Zbuf)

            # ---- fused B -> mix -> T per chunk of HC h' rows ----
            # merged planes: Wpl[:, 0] = real, Wpl[:, 1] = imag
            Wpl = planep.tile([WF, 2, H, BS], bf, tag="wpl")
            for ch_i in range(H // HC):
                h0 = ch_i * HC
                arch = mixp.tile([BS, 2, HC, WF], f8, tag="arch", bufs=4)
                for j2 in range(HC // 2):
                    pB = psum.tile([BS, 2, 2 * WF], f32, tag="ps_b")
                    for j in range(2):
                        hj = h0 + j2 * 2 + j
                        nc.tensor.matmul(
                            pB[:, j, :], lhsT=Zbuf[:, :, :, hj], rhs=cBp_sb,
                            start=True, stop=True, perf_mode=DRow,
                        )
                    pBr = pB.rearrange("p j (r f) -> p r j f", r=2)
                    if j2 == 0:
                        nc.scalar.copy(arch[:, :, 0:2, :], pBr)
                    else:
                        nc.vector.tensor_copy(arch[:, :, 2:4, :], pBr)
                # layer 1 (DoubleRow: k-tiles = (Ar, Ai))
                p1r = psum.tile([BS, HC, WF], f32, tag="ps_m", bufs=3)
                nc.tensor.matmul(p1r, lhsT=w1pr_sb[:, :, n, :], rhs=arch,
                                 start=True, stop=True, perf_mode=DRow)
                p1i = psum.tile([BS, HC, WF], f32, tag="ps_m", bufs=3)
                nc.tensor.matmul(p1i, lhsT=w1pi_sb[:, :, n, :], rhs=arch,
                                 start=True, stop=True, perf_mode=DRow)
                r1 = mixp.tile([BS, HC, WF], bf, tag="r1", bufs=4)
                i1 = mixp.tile([BS, HC, WF], bf, tag="i1", bufs=4)
                nc.scalar.activation(
                    r1, p1r, AF.Relu, bias=addr_v[:, n: n + 1],
                    scale=shp1_s[:, n: n + 1]
                )
                nc.scalar.activation(
                    i1, p1i, AF.Relu, bias=addi_v[:, n: n + 1],
                    scale=shp1_s[:, n: n + 1]
                )
                # layer 2: r2 = w2r@r1 - w2i@i1 + b2r ; i2 = A2@r1 + B2@i1 + c2
                p2r = psum.tile([BS, HC, WF], f32, tag="ps_m", bufs=3)
                nc.tensor.matmul(p2r, lhsT=w2r_sb[:, n, :], rhs=r1, start=True, stop=False)
                nc.tensor.matmul(p2r, lhsT=nw2i_sb[:, n, :], rhs=i1, start=False, stop=True)
                p2i = psum.tile([BS, HC, WF], f32, tag="ps_m", bufs=3)
                nc.tensor.matmul(p2i, lhsT=w2A_sb[:, n, :], rhs=r1, start=True, stop=False)
                nc.tensor.matmul(p2i, lhsT=w2B_sb[:, n, :], rhs=i1, start=False, stop=True)
                # biased r2|i2 staged in one tile: rb2[:, 0] = r2, rb2[:, 1] = i2
                rb2 = mixp.tile([BS, 2, HC, WF], bf, tag="rb2", bufs=4)
                nc.scalar.activation(rb2[:, 0, :, :], p2r, AF.Identity,
                                     bias=b2r_v[:, n: n + 1])
                nc.vector.tensor_scalar(rb2[:, 1, :, :], p2i, c2_v[:, n: n + 1],
                                        None, ALU.add)
                # softshrink(v) = v - clip(v, -lam, lam): clip on Pool, sub on DVE
                sab = mixp.tile([BS, 2, HC, WF], bf, tag="sab", bufs=4)
                nc.gpsimd.tensor_scalar(sab, rb2, -LAM, LAM, ALU.max, ALU.min)
                R2I2 = mixp.tile([BS, 2, HC, WF], bf, tag="R2I2", bufs=4)
                nc.vector.tensor_sub(R2I2[:, 0, :, :], rb2[:, 0, :, :],
                                     sab[:, 0, :, :])
                nc.gpsimd.tensor_sub(R2I2[:, 1, :, :], rb2[:, 1, :, :],
                                     sab[:, 1, :, :])
                # T: pivot [c, wf] -> [wf, c]; one merged psum bank, one evict
                pT = psum.tile([WF, 2, HC, BS], bf, tag="ps_t", bufs=1)
                for j in range(HC):
                    nc.tensor.transpose(pT[:, 0, j, :], R2I2[:, 0, j, :],
                                        cI_sb[0:BS, 0:BS])
                    nc.tensor.transpose(pT[:, 1, j, :], R2I2[:, 1, j, :],
                                        cI_sb[0:BS, 0:BS])
                nc.vector.tensor_copy(Wpl[:, :, h0: h0 + HC, :], pT)

            # ---- stages E' + D' in sub-groups of CSUB channels ----
            for sub in range(BS // CSUB):
                cb = sub * CSUB
                Pbuf = pbufp.tile([H, CSUB, 2 * H], bf, tag="pbuf")
                for cp in range(CSUB // 2):
                    c = cb + 2 * cp
                    if n == NBL - 1:
                        # drain: mix + A/B rings are idle after the last mix
                        if cp % 2 == 0:
                            pE = psum.tile([128, 2, 2 * H], f32, tag="ps_m", bufs=3)
                        else:
                            pE = psum.tile([128, 2, 2 * H], f32, tag="ps_b")
                    else:
                        pE = psum.tile([128, 2, 2 * H], f32, tag="ps_a")
                    for q in range(2):
                        nc.tensor.matmul(
                            pE[:, q, :], lhsT=Wpl[:, 0, :, c + q], rhs=cE1_sb,
                            start=True, stop=False,
                        )
                        nc.tensor.matmul(
                            pE[:, q, :], lhsT=Wpl[:, 1, :, c + q], rhs=cE2_sb,
                            start=False, stop=True,
                        )
                    if cp % 2 == 0:
                        nc.vector.tensor_copy(Pbuf[:, 2 * cp: 2 * cp + 2, :], pE)
                    else:
                        nc.scalar.copy(Pbuf[:, 2 * cp: 2 * cp + 2, :], pE)
                # D': out = FHr@Pr - FHi@Pi + x
                for g in range(CSUB // 4):
                    gc = 4 * g
                    pD = psum.tile([H, 4, W], f32, tag="ps_a")
                    nc.tensor.matmul(
                        pD, lhsT=cDr_sb, rhs=Pbuf[:, gc: gc + 4, 0:H],
                        start=True, stop=False,
                    )
                    ot = outp.tile([H, 4, W], bf, tag="ot")
                    if n != NBL - 1 and g % 3 != 2:
                        # residual folded into the DVE eviction (same cost as copy)
                        nc.tensor.matmul(
                            pD, lhsT=cDi_sb, rhs=Pbuf[:, gc: gc + 4, H: 2 * H],
                            start=False, stop=True,
                        )
                        nc.vector.tensor_add(ot, pD, X16[:, cb + gc: cb + gc + 4, :])
                    else:
                        # ACT can't tensor+tensor: accumulate x via identity matmul
                        nc.tensor.matmul(
                            pD, lhsT=cDi_sb, rhs=Pbuf[:, gc: gc + 4, H: 2 * H],
                            start=False, stop=False,
                        )
                        nc.tensor.matmul(
                            pD, lhsT=cI_sb, rhs=X16[:, cb + gc: cb + gc + 4, :],
                            start=False, stop=True,
                        )
                        nc.scalar.copy(ot, pD)
                    nc.sync.dma_start(
                        outs[:, c0 + cb + gc: c0 + cb + gc + 4, :], ot
                    )

    nc.compile()
    return nc


_CACHE = {}


def _get_program():
    if "nc" not in _CACHE:
        _CACHE["nc"] = _build_program()
    return _CACHE["nc"]


def kernel(**inputs):
    x = np.asarray(inputs["x"], dtype=np.float32)
    t = np.asarray(inputs["t"], dtype=np.float32)
    w1 = np.asarray(inputs["w1"], dtype=np.float32)
    b1 = np.asarray(inputs["b1"], dtype=np.float32)
    w2 = np.asarray(inputs["w2"], dtype=np.float32)
    b2 = np.asarray(inputs["b2"], dtype=np.float32)
    mod_w = np.asarray(inputs["mod_w"], dtype=np.float32)
    mod_b = np.asarray(inputs["mod_b"], dtype=np.float32)

    from concourse.bass_utils import run_bass_kernel_spmd

    nc = _get_program()
    consts = _host_consts()

    silu_t = t / (1.0 + np.exp(-t))
    mod_full = silu_t @ mod_w.T + mod_b            # (B, 2*DIM)
    mod_full = mod_full.reshape(B_FULL, NB, 2 * BS)
    in_maps = []
    for core in range(N_CORES):
        b = core // 2
        sh = mod_full[b, :, :BS] + 1.0             # (NB, BS)
        sc = mod_full[b, :, BS:]
        n0 = (core % 2) * NBL
        cs = slice(n0 * BS, n0 * BS + C)
        rs = slice(n0 * 2 * BS, (n0 + NBL) * 2 * BS)
        W1 = w1[:, n0: n0 + NBL]          # [2, NBL, BS, BS] (d, k)
        W2 = w2[:, n0: n0 + NBL]
        im = {
            "xhw": np.ascontiguousarray(
                x[b, cs].transpose(1, 0, 2)).astype(BF16),          # [h, c, w]
            "w1pr": np.ascontiguousarray(np.stack(
                [8.0 * W1[0], -8.0 * W1[1]], 0).transpose(2, 0, 1, 3)).astype(F8),
            "w1pi": np.ascontiguousarray(np.stack(
                [8.0 * W1[1], 8.0 * W1[0]], 0).transpose(2, 0, 1, 3)).astype(F8),
            "w2r": np.ascontiguousarray(W2[0].transpose(1, 0, 2)).astype(BF16),
            "w2i": np.ascontiguousarray(W2[1].transpose(1, 0, 2)).astype(BF16),
            "nw2i": np.ascontiguousarray(-W2[1].transpose(1, 0, 2)).astype(BF16),
            "w2Ad": np.ascontiguousarray(
                np.einsum("nde,nek->dnk", W2[0], W2[1])).astype(BF16),
            "w2Bd": np.ascontiguousarray(
                (W2[0] - np.einsum("nde,nek->ndk", W2[1], W2[1])
                 ).transpose(1, 0, 2)).astype(BF16),
            "c2d": np.ascontiguousarray(
                (np.einsum("nd,ndk->nk", b2[0, n0: n0 + NBL], W2[1])
                 + b2[1, n0: n0 + NBL]).T).astype(np.float32),
            "shp1d": np.ascontiguousarray(sh[n0: n0 + NBL].T),
            "shp1sd": np.ascontiguousarray(sh[n0: n0 + NBL].T / 128.0),
            "addrd": np.ascontiguousarray(
                (b1[0, n0: n0 + NBL] * sh[n0: n0 + NBL]
                 + sc[n0: n0 + NBL]).T),
            "addid": np.ascontiguousarray(
                (b1[1, n0: n0 + NBL] * sh[n0: n0 + NBL]
                 + sc[n0: n0 + NBL]).T),
            "b2rd": np.ascontiguousarray(b2[0, n0: n0 + NBL].T),
            "b2id": np.ascontiguousarray(b2[1, n0: n0 + NBL].T),
        }
        im.update(consts)
        in_maps.append(im)

    res = run_bass_kernel_spmd(
        nc, in_maps, core_ids=list(range(N_CORES))
    )

    out = np.empty((B_FULL, DIM, H, W), dtype=np.float32)
    for core in range(N_CORES):
        b = core // 2
        n0 = (core % 2) * NBL
        cs = slice(n0 * BS, n0 * BS + C)
        out[b, cs] = res.results[core]["outs"].astype(np.float32).transpose(1, 0, 2)
    return out



# revision 18
# speedup vs baseline: 1.0078x; 1.0078x over previous
"""ModAFNO2D layer as a Bass/Tile kernel for 8 Trainium2 NeuronCores.

Sharding: 8 cores = (batch b in 0..3) x (block-half in 0..1). Each core owns one
batch sample and 4 of the 8 FNO blocks (= 384 of 768 channels). The FFT axes are
per-channel and channel blocks never mix, so cores are fully independent — no
collectives; host slices inputs and concatenates outputs.

Per-core pipeline (all heavy math on the PE as bf16 matmuls; DFTs as matrix
multiplies with precomputed 128-point DFT matrices):
  A : Z^T = X_c^T @ [Fr|Fi]                 (FFT along H; X_c stationary)
  B : Y[c,(Yr|Yi)] at fixed h'              (rFFT along W)
  mix: block-diagonal 2-layer complex MLP with adaLN modulation. The second
       layer's imag output is rewritten as i2 = A2@r1 + B2@i1 + c2 with
       A2 = w2r@w2i, B2 = w2r - w2i@w2i, c2 = b2r@w2i + b2i (computed once on
       device), which removes the r2->i2 serial dependency.
  T : PE transposes [c,wf]->[wf,c] to pivot back to spatial-major
  E': [Pr|Pi] = Z @ [Sr|Si]                 (inverse rFFT along W)
  D': out = FHr@Pr - FHi@Pi + x             (inverse FFT along H + residual)
All spectra/activations bf16 (PSUM accumulation fp32); input x is shipped
pre-transposed [h, c, w] in bf16 and the residual/output stay bf16 (host
casts back to fp32). The residual add rides the last D' matmul (identity
stationary). PSUM evictions are spread over DVE/ACT; the SBUF-only softshrink
clip/sub ops run on GpSimd (Pool); all DMAs issue from the otherwise-idle SP
queue. Block-0's X prefetch and stage A are hoisted ahead of the one-time
setup (modulation, W2 combos) so the PE starts ~5us into the program.
"""

import numpy as np
import ml_dtypes

BF16 = ml_dtypes.bfloat16
F8 = ml_dtypes.float8_e4m3

DIM = 768
NB = 8
BS = 96
LAM = 0.01
B_FULL = 4
H = 128
W = 128
WF = W // 2 + 1  # 65
NBL = 4          # blocks per core
C = NBL * BS     # 384 channels per core
N_CORES = 8
HC = 4           # h' rows per fused B/mix/T chunk
CSUB = 24        # channels per E/D sub-group (Pbuf ring tile)


def _host_consts():
    jh = np.arange(H)
    F = np.exp(-2j * np.pi * np.outer(jh, jh) / H)
    R = np.exp(-2j * np.pi * np.outer(np.arange(WF), np.arange(W)) / W) / 128.0
    RrT, RiT = R.real.T, R.imag.T                      # [w, wf]
    FH = np.conj(F)
    cw = np.ones(WF)
    cw[1:-1] = 2.0
    S = (cw[:, None] * np.exp(2j * np.pi * np.outer(np.arange(WF), np.arange(W)) / W)) / 128.0
    consts = {
        "cF": np.concatenate([F.real, F.imag], 1).astype(BF16),            # [128, 256]
        "cBp": (16.0 * np.stack(
            [np.concatenate([RrT, RiT], 1),
             np.concatenate([-RiT, RrT], 1)], axis=1)).astype(F8),         # [128, 2, 130]
        "cE1": np.concatenate([S.real, S.imag], 1).astype(BF16),           # [65, 256]
        "cE2": np.concatenate([-S.imag, S.real], 1).astype(BF16),          # [65, 256]
        "cDr": FH.real.astype(BF16),                                       # [128, 128]
        "cDi": (-FH.imag).astype(BF16),                                    # [128, 128]
        "cI": np.eye(128, dtype=np.float32).astype(BF16),                  # [128, 128]
    }
    return consts


def _build_program():
    from contextlib import ExitStack

    import concourse.bass as bass  # noqa: F401
    import concourse.mybir as mybir
    import concourse.tile as tile
    from concourse import bacc

    f32 = mybir.dt.float32
    bf = mybir.dt.bfloat16
    f8 = mybir.dt.float8e4
    DRow = mybir.MatmulPerfMode.DoubleRow
    AF = mybir.ActivationFunctionType
    ALU = mybir.AluOpType

    nc = bacc.Bacc("TRN2", target_bir_lowering=False, debug=False)

    xhw = nc.dram_tensor("xhw", [H, C, W], bf, kind="ExternalInput")
    w1pr = nc.dram_tensor("w1pr", [BS, 2, NBL, BS], f8, kind="ExternalInput")
    w1pi = nc.dram_tensor("w1pi", [BS, 2, NBL, BS], f8, kind="ExternalInput")
    w2r = nc.dram_tensor("w2r", [BS, NBL, BS], bf, kind="ExternalInput")
    w2i = nc.dram_tensor("w2i", [BS, NBL, BS], bf, kind="ExternalInput")
    nw2i = nc.dram_tensor("nw2i", [BS, NBL, BS], bf, kind="ExternalInput")
    w2Ad = nc.dram_tensor("w2Ad", [BS, NBL, BS], bf, kind="ExternalInput")
    w2Bd = nc.dram_tensor("w2Bd", [BS, NBL, BS], bf, kind="ExternalInput")
    c2d = nc.dram_tensor("c2d", [BS, NBL], f32, kind="ExternalInput")
    shp1d = nc.dram_tensor("shp1d", [BS, NBL], f32, kind="ExternalInput")
    shp1sd = nc.dram_tensor("shp1sd", [BS, NBL], f32, kind="ExternalInput")
    addrd = nc.dram_tensor("addrd", [BS, NBL], f32, kind="ExternalInput")
    addid = nc.dram_tensor("addid", [BS, NBL], f32, kind="ExternalInput")
    b2rd = nc.dram_tensor("b2rd", [BS, NBL], f32, kind="ExternalInput")
    b2id = nc.dram_tensor("b2id", [BS, NBL], f32, kind="ExternalInput")
    cF = nc.dram_tensor("cF", [H, 2 * H], bf, kind="ExternalInput")
    cBp = nc.dram_tensor("cBp", [W, 2, 2 * WF], f8, kind="ExternalInput")
    cE1 = nc.dram_tensor("cE1", [WF, 2 * W], bf, kind="ExternalInput")
    cE2 = nc.dram_tensor("cE2", [WF, 2 * W], bf, kind="ExternalInput")
    cDr = nc.dram_tensor("cDr", [H, H], bf, kind="ExternalInput")
    cDi = nc.dram_tensor("cDi", [H, H], bf, kind="ExternalInput")
    cI = nc.dram_tensor("cI", [128, 128], bf, kind="ExternalInput")
    outs = nc.dram_tensor("outs", [H, C, W], bf, kind="ExternalOutput")

    with ExitStack() as ctx:
        tc = ctx.enter_context(tile.TileContext(nc))
        consts = ctx.enter_context(tc.tile_pool(name="consts", bufs=1))
        xpool = ctx.enter_context(tc.tile_pool(name="xpool", bufs=2))
        zpool = ctx.enter_context(tc.tile_pool(name="zpool", bufs=1))
        planep = ctx.enter_context(tc.tile_pool(name="planep", bufs=1))
        pbufp = ctx.enter_context(tc.tile_pool(name="pbufp", bufs=3))
        mixp = ctx.enter_context(tc.tile_pool(name="mixp", bufs=2))
        outp = ctx.enter_context(tc.tile_pool(name="outp", bufs=5))
        psum = ctx.enter_context(tc.tile_pool(name="psum", bufs=2, space="PSUM"))

        # ---- stage-A DFT matrix first, then block-0 X prefetch (SP queue) ----
        cF_sb = consts.tile([H, 2 * H], bf)
        nc.sync.dma_start(cF_sb, cF[:])
        X16_first = xpool.tile([H, BS, W], bf, tag="xblk")
        for cc in range(0, BS, CSUB):
            nc.sync.dma_start(
                X16_first[:, cc: cc + CSUB, :], xhw[:, cc: cc + CSUB, :]
            )
        # ---- modulation vectors computed on host: shp1=shift+1, shp1_s=shp1/128,
        # addv = b1*shp1 + scale (ships 4 tiny f32 vectors instead of mwT) ----
        shp1 = consts.tile([BS, NBL], f32)
        shp1_s = consts.tile([BS, NBL], f32)
        addr_v = consts.tile([BS, NBL], f32)
        addi_v = consts.tile([BS, NBL], f32)
        nc.sync.dma_start(shp1, shp1d[:])
        nc.sync.dma_start(shp1_s, shp1sd[:])
        nc.sync.dma_start(addr_v, addrd[:])
        nc.sync.dma_start(addi_v, addid[:])

        cBp_sb = consts.tile([W, 2, 2 * WF], f8)
        nc.sync.dma_start(cBp_sb, cBp[:])
        cE1_sb = consts.tile([WF, 2 * W], bf)
        nc.sync.dma_start(cE1_sb, cE1[:])
        cE2_sb = consts.tile([WF, 2 * W], bf)
        nc.sync.dma_start(cE2_sb, cE2[:])
        cDr_sb = consts.tile([H, H], bf)
        nc.sync.dma_start(cDr_sb, cDr[:])
        cDi_sb = consts.tile([H, H], bf)
        nc.sync.dma_start(cDi_sb, cDi[:])
        cI_sb = consts.tile([128, 128], bf)
        nc.sync.dma_start(cI_sb, cI[:])

        # ---- block weights ----
        w1pr_sb = consts.tile([BS, 2, NBL, BS], f8)
        w1pi_sb = consts.tile([BS, 2, NBL, BS], f8)
        w2r_sb = consts.tile([BS, NBL, BS], bf)
        w2i_sb = consts.tile([BS, NBL, BS], bf)
        nw2i_sb = consts.tile([BS, NBL, BS], bf)
        w2A_sb = consts.tile([BS, NBL, BS], bf)   # w2r @ w2i (host)
        w2B_sb = consts.tile([BS, NBL, BS], bf)   # w2r - w2i @ w2i (host)
        for t_sb_, t_dr_ in ((w1pr_sb, w1pr), (w1pi_sb, w1pi),
                             (w2r_sb, w2r), (w2i_sb, w2i), (nw2i_sb, nw2i),
                             (w2A_sb, w2Ad), (w2B_sb, w2Bd)):
            nc.sync.dma_start(t_sb_, t_dr_[:])
        b2r_v = consts.tile([BS, NBL], f32)
        c2_v = consts.tile([BS, NBL], f32)        # b2r @ w2i + b2i (host)
        nc.sync.dma_start(b2r_v, b2rd[:])
        nc.sync.dma_start(c2_v, c2d[:])

        def stage_a(X16, Zbuf, deep=False):
            for cp in range(BS // 2):
                c = 2 * cp
                if deep and cp % 2 == 1:
                    # startup only: other rings are idle, deepen the pipeline
                    pA = psum.tile([128, 2, 2 * H], f32, tag="ps_m", bufs=3)
                else:
                    pA = psum.tile([128, 2, 2 * H], f32, tag="ps_b")
                nc.tensor.matmul(pA[:, 0, :], lhsT=X16[:, c, :], rhs=cF_sb,
                                 start=True, stop=True)
                nc.tensor.matmul(pA[:, 1, :], lhsT=X16[:, c + 1, :], rhs=cF_sb,
                                 start=True, stop=True)
                pAr = pA.rearrange("w c (r h) -> w r c h", r=2)
                if cp % 2 == 0:
                    nc.vector.tensor_copy(Zbuf[:, :, c: c + 2, :], pAr)
                else:
                    nc.scalar.copy(Zbuf[:, :, c: c + 2, :], pAr)

        # stage A of block 0 runs before the one-time setup sections so the
        # PE starts as soon as the first X chunk lands
        Zbuf_first = zpool.tile([W, 2, BS, H], f8, tag="zbuf")
        stage_a(X16_first, Zbuf_first, deep=True)



        # ---- main per-block pipeline ----
        for n in range(NBL):
            c0 = n * BS

            # resident X for this block: [h, c, w] bf16 (stage-A lhsT + residual)
            if n == 0:
                X16 = X16_first
            else:
                X16 = xpool.tile([H, BS, W], bf, tag="xblk")
                for cc in range(0, BS, CSUB):
                    nc.sync.dma_start(
                        X16[:, cc: cc + CSUB, :],
                        xhw[:, c0 + cc: c0 + cc + CSUB, :],
                    )

            # ---- stage A: Z^T = X_c^T @ [Fr|Fi] -> Zbuf [w, c, h'Zr|h'Zi] ----
            if n == 0:
                Zbuf = Zbuf_first
            else:
                Zbuf = zpool.tile([W, 2, BS, H], f8, tag="zbuf")
                stage_a(X16, Zbuf)

            # ---- fused B -> mix -> T per chunk of HC h' rows ----
            # merged planes: Wpl[:, 0] = real, Wpl[:, 1] = imag
            Wpl = planep.tile([WF, 2, H, BS], bf, tag="wpl")
            for ch_i in range(H // HC):
                h0 = ch_i * HC
                arch = mixp.tile([BS, 2, HC, WF], f8, tag="arch", bufs=4)
                for j2 in range(HC // 2):
                    pB = psum.tile([BS, 2, 2 * WF], f32, tag="ps_b")
                    for j in range(2):
                        hj = h0 + j2 * 2 + j
                        nc.tensor.matmul(
                            pB[:, j, :], lhsT=Zbuf[:, :, :, hj], rhs=cBp_sb,
                            start=True, stop=True, perf_mode=DRow,
                        )
                    pBr = pB.rearrange("p j (r f) -> p r j f", r=2)
                    if j2 == 0:
                        nc.scalar.copy(arch[:, :, 0:2, :], pBr)
                    else:
                        nc.vector.tensor_copy(arch[:, :, 2:4, :], pBr)
                # layer 1 (DoubleRow: k-tiles = (Ar, Ai))
                p1r = psum.tile([BS, HC, WF], f32, tag="ps_m", bufs=3)
                nc.tensor.matmul(p1r, lhsT=w1pr_sb[:, :, n, :], rhs=arch,
                                 start=True, stop=True, perf_mode=DRow)
                p1i = psum.tile([BS, HC, WF], f32, tag="ps_m", bufs=3)
                nc.tensor.matmul(p1i, lhsT=w1pi_sb[:, :, n, :], rhs=arch,
                                 start=True, stop=True, perf_mode=DRow)
                r1 = mixp.tile([BS, HC, WF], bf, tag="r1", bufs=4)
                i1 = mixp.tile([BS, HC, WF], bf, tag="i1", bufs=4)
                nc.scalar.activation(
                    r1, p1r, AF.Relu, bias=addr_v[:, n: n + 1],
                    scale=shp1_s[:, n: n + 1]
                )
                nc.scalar.activation(
                    i1, p1i, AF.Relu, bias=addi_v[:, n: n + 1],
                    scale=shp1_s[:, n: n + 1]
                )
                # layer 2: r2 = w2r@r1 - w2i@i1 + b2r ; i2 = A2@r1 + B2@i1 + c2
                p2r = psum.tile([BS, HC, WF], f32, tag="ps_m", bufs=3)
                nc.tensor.matmul(p2r, lhsT=w2r_sb[:, n, :], rhs=r1, start=True, stop=False)
                nc.tensor.matmul(p2r, lhsT=nw2i_sb[:, n, :], rhs=i1, start=False, stop=True)
                p2i = psum.tile([BS, HC, WF], f32, tag="ps_m", bufs=3)
                nc.tensor.matmul(p2i, lhsT=w2A_sb[:, n, :], rhs=r1, start=True, stop=False)
                nc.tensor.matmul(p2i, lhsT=w2B_sb[:, n, :], rhs=i1, start=False, stop=True)
                # biased r2|i2 staged in one tile: rb2[:, 0] = r2, rb2[:, 1] = i2
                rb2 = mixp.tile([BS, 2, HC, WF], bf, tag="rb2", bufs=4)
                nc.scalar.activation(rb2[:, 0, :, :], p2r, AF.Identity,
                                     bias=b2r_v[:, n: n + 1])
                nc.vector.tensor_scalar(rb2[:, 1, :, :], p2i, c2_v[:, n: n + 1],
                                        None, ALU.add)
                # softshrink(v) = v - clip(v, -lam, lam): clip on Pool, sub on DVE
                sab = mixp.tile([BS, 2, HC, WF], bf, tag="sab", bufs=4)
                nc.gpsimd.tensor_scalar(sab, rb2, -LAM, LAM, ALU.max, ALU.min)
                R2I2 = mixp.tile([BS, 2, HC, WF], bf, tag="R2I2", bufs=4)
                nc.vector.tensor_sub(R2I2[:, 0, :, :], rb2[:, 0, :, :],
                                     sab[:, 0, :, :])
                nc.gpsimd.tensor_sub(R2I2[:, 1, :, :], rb2[:, 1, :, :],
                                     sab[:, 1, :, :])
                # T: pivot [c, wf] -> [wf, c]; one merged psum bank, one evict
                pT = psum.tile([WF, 2, HC, BS], bf, tag="ps_t", bufs=1)
                for j in range(HC):
                    nc.tensor.transpose(pT[:, 0, j, :], R2I2[:, 0, j, :],
                                        cI_sb[0:BS, 0:BS])
                    nc.tensor.transpose(pT[:, 1, j, :], R2I2[:, 1, j, :],
                                        cI_sb[0:BS, 0:BS])
                nc.vector.tensor_copy(Wpl[:, :, h0: h0 + HC, :], pT)

            # ---- stages E' + D' in sub-groups of CSUB channels ----
            for sub in range(BS // CSUB):
                cb = sub * CSUB
                Pbuf = pbufp.tile([H, CSUB, 2 * H], bf, tag="pbuf")
                for cp in range(CSUB // 2):
                    c = cb + 2 * cp
                    if n == NBL - 1:
                        # drain: mix + A/B rings are idle after the last mix
                        if cp % 2 == 0:
                            pE = psum.tile([128, 2, 2 * H], f32, tag="ps_m", bufs=3)
                        else:
                            pE = psum.tile([128, 2, 2 * H], f32, tag="ps_b")
                    else:
                        pE = psum.tile([128, 2, 2 * H], f32, tag="ps_a")
                    for q in range(2):
                        nc.tensor.matmul(
                            pE[:, q, :], lhsT=Wpl[:, 0, :, c + q], rhs=cE1_sb,
                            start=True, stop=False,
                        )
                        nc.tensor.matmul(
                            pE[:, q, :], lhsT=Wpl[:, 1, :, c + q], rhs=cE2_sb,
                            start=False, stop=True,
                        )
                    if cp % 2 == 0:
                        nc.vector.tensor_copy(Pbuf[:, 2 * cp: 2 * cp + 2, :], pE)
                    else:
                        nc.scalar.copy(Pbuf[:, 2 * cp: 2 * cp + 2, :], pE)
                # D': out = FHr@Pr - FHi@Pi + x
                for g in range(CSUB // 4):
                    gc = 4 * g
                    pD = psum.tile([H, 4, W], f32, tag="ps_a")
                    nc.tensor.matmul(
                        pD, lhsT=cDr_sb, rhs=Pbuf[:, gc: gc + 4, 0:H],
                        start=True, stop=False,
                    )
                    ot = outp.tile([H, 4, W], bf, tag="ot")
                    if g % 3 != 2:
                        # residual folded into the DVE eviction (same cost as copy)
                        nc.tensor.matmul(
                            pD, lhsT=cDi_sb, rhs=Pbuf[:, gc: gc + 4, H: 2 * H],
                            start=False, stop=True,
                        )
                        nc.vector.tensor_add(ot, pD, X16[:, cb + gc: cb + gc + 4, :])
                    else:
                        # ACT can't tensor+tensor: accumulate x via identity matmul
                        nc.tensor.matmul(
                            pD, lhsT=cDi_sb, rhs=Pbuf[:, gc: gc + 4, H: 2 * H],
                            start=False, stop=False,
                        )
                        nc.tensor.matmul(
                            pD, lhsT=cI_sb, rhs=X16[:, cb + gc: cb + gc + 4, :],
                            start=False, stop=True,
                        )
                        nc.scalar.copy(ot, pD)
                    nc.sync.dma_start(
                        outs[:, c0 + cb + gc: c0 + cb + gc + 4, :], ot
                    )

    nc.compile()
    return nc


_CACHE = {}


def _get_program():
    if "nc" not in _CACHE:
        _CACHE["nc"] = _build_program()
    return _CACHE["nc"]


def kernel(**inputs):
    x = np.asarray(inputs["x"], dtype=np.float32)
    t = np.asarray(inputs["t"], dtype=np.float32)
    w1 = np.asarray(inputs["w1"], dtype=np.float32)
    b1 = np.asarray(inputs["b1"], dtype=np.float32)
    w2 = np.asarray(inputs["w2"], dtype=np.float32)
    b2 = np.asarray(inputs["b2"], dtype=np.float32)
    mod_w = np.asarray(inputs["mod_w"], dtype=np.float32)
    mod_b = np.asarray(inputs["mod_b"], dtype=np.float32)

    from concourse.bass_utils import run_bass_kernel_spmd

    nc = _get_program()
    consts = _host_consts()

    silu_t = t / (1.0 + np.exp(-t))
    mod_full = silu_t @ mod_w.T + mod_b            # (B, 2*DIM)
    mod_full = mod_full.reshape(B_FULL, NB, 2 * BS)
    in_maps = []
    for core in range(N_CORES):
        b = core // 2
        sh = mod_full[b, :, :BS] + 1.0             # (NB, BS)
        sc = mod_full[b, :, BS:]
        n0 = (core % 2) * NBL
        cs = slice(n0 * BS, n0 * BS + C)
        rs = slice(n0 * 2 * BS, (n0 + NBL) * 2 * BS)
        W1 = w1[:, n0: n0 + NBL]          # [2, NBL, BS, BS] (d, k)
        W2 = w2[:, n0: n0 + NBL]
        im = {
            "xhw": np.ascontiguousarray(
                x[b, cs].transpose(1, 0, 2)).astype(BF16),          # [h, c, w]
            "w1pr": np.ascontiguousarray(np.stack(
                [8.0 * W1[0], -8.0 * W1[1]], 0).transpose(2, 0, 1, 3)).astype(F8),
            "w1pi": np.ascontiguousarray(np.stack(
                [8.0 * W1[1], 8.0 * W1[0]], 0).transpose(2, 0, 1, 3)).astype(F8),
            "w2r": np.ascontiguousarray(W2[0].transpose(1, 0, 2)).astype(BF16),
            "w2i": np.ascontiguousarray(W2[1].transpose(1, 0, 2)).astype(BF16),
            "nw2i": np.ascontiguousarray(-W2[1].transpose(1, 0, 2)).astype(BF16),
            "w2Ad": np.ascontiguousarray(
                np.einsum("nde,nek->dnk", W2[0], W2[1])).astype(BF16),
            "w2Bd": np.ascontiguousarray(
                (W2[0] - np.einsum("nde,nek->ndk", W2[1], W2[1])
                 ).transpose(1, 0, 2)).astype(BF16),
            "c2d": np.ascontiguousarray(
                (np.einsum("nd,ndk->nk", b2[0, n0: n0 + NBL], W2[1])
                 + b2[1, n0: n0 + NBL]).T).astype(np.float32),
            "shp1d": np.ascontiguousarray(sh[n0: n0 + NBL].T),
            "shp1sd": np.ascontiguousarray(sh[n0: n0 + NBL].T / 128.0),
            "addrd": np.ascontiguousarray(
                (b1[0, n0: n0 + NBL] * sh[n0: n0 + NBL]
                 + sc[n0: n0 + NBL]).T),
            "addid": np.ascontiguousarray(
                (b1[1, n0: n0 + NBL] * sh[n0: n0 + NBL]
                 + sc[n0: n0 + NBL]).T),
            "b2rd": np.ascontiguousarray(b2[0, n0: n0 + NBL].T),
            "b2id": np.ascontiguousarray(b2[1, n0: n0 + NBL].T),
        }
        im.update(consts)
        in_maps.append(im)

    res = run_bass_kernel_spmd(
        nc, in_maps, core_ids=list(range(N_CORES))
    )

    out = np.empty((B_FULL, DIM, H, W), dtype=np.float32)
    for core in range(N_CORES):
        b = core // 2
        n0 = (core % 2) * NBL
        cs = slice(n0 * BS, n0 * BS + C)
        out[b, cs] = res.results[core]["outs"].astype(np.float32).transpose(1, 0, 2)
    return out



# revision 19
# speedup vs baseline: 1.0129x; 1.0050x over previous
"""ModAFNO2D layer as a Bass/Tile kernel for 8 Trainium2 NeuronCores.

Sharding: 8 cores = (batch b in 0..3) x (block-half in 0..1). Each core owns one
batch sample and 4 of the 8 FNO blocks (= 384 of 768 channels). The FFT axes are
per-channel and channel blocks never mix, so cores are fully independent — no
collectives; host slices inputs and concatenates outputs.

Per-core pipeline (all heavy math on the PE as bf16 matmuls; DFTs as matrix
multiplies with precomputed 128-point DFT matrices):
  A : Z^T = X_c^T @ [Fr|Fi]                 (FFT along H; X_c stationary)
  B : Y[c,(Yr|Yi)] at fixed h'              (rFFT along W)
  mix: block-diagonal 2-layer complex MLP with adaLN modulation. The second
       layer's imag output is rewritten as i2 = A2@r1 + B2@i1 + c2 with
       A2 = w2r@w2i, B2 = w2r - w2i@w2i, c2 = b2r@w2i + b2i (computed once on
       device), which removes the r2->i2 serial dependency.
  T : PE transposes [c,wf]->[wf,c] to pivot back to spatial-major
  E': [Pr|Pi] = Z @ [Sr|Si]                 (inverse rFFT along W)
  D': out = FHr@Pr - FHi@Pi + x             (inverse FFT along H + residual)
All spectra/activations bf16 (PSUM accumulation fp32); input x is shipped
pre-transposed [h, c, w] in bf16 and the residual/output stay bf16 (host
casts back to fp32). The residual add rides the last D' matmul (identity
stationary). PSUM evictions are spread over DVE/ACT; the SBUF-only softshrink
clip/sub ops run on GpSimd (Pool); all DMAs issue from the otherwise-idle SP
queue. Block-0's X prefetch and stage A are hoisted ahead of the one-time
setup (modulation, W2 combos) so the PE starts ~5us into the program.
"""

import numpy as np
import ml_dtypes

BF16 = ml_dtypes.bfloat16
F8 = ml_dtypes.float8_e4m3

DIM = 768
NB = 8
BS = 96
LAM = 0.01
B_FULL = 4
H = 128
W = 128
WF = W // 2 + 1  # 65
NBL = 4          # blocks per core
C = NBL * BS     # 384 channels per core
N_CORES = 8
HC = 4           # h' rows per fused B/mix/T chunk
CSUB = 24        # channels per E/D sub-group (Pbuf ring tile)


def _host_consts():
    jh = np.arange(H)
    F = np.exp(-2j * np.pi * np.outer(jh, jh) / H)
    R = np.exp(-2j * np.pi * np.outer(np.arange(WF), np.arange(W)) / W) / 128.0
    RrT, RiT = R.real.T, R.imag.T                      # [w, wf]
    FH = np.conj(F)
    cw = np.ones(WF)
    cw[1:-1] = 2.0
    S = (cw[:, None] * np.exp(2j * np.pi * np.outer(np.arange(WF), np.arange(W)) / W)) / 128.0
    consts = {
        "cF": np.concatenate([F.real, F.imag], 1).astype(BF16),            # [128, 256]
        "cBp": (16.0 * np.stack(
            [np.concatenate([RrT, RiT], 1),
             np.concatenate([-RiT, RrT], 1)], axis=1)).astype(F8),         # [128, 2, 130]
        "cE1": np.concatenate([S.real, S.imag], 1).astype(BF16),           # [65, 256]
        "cE2": np.concatenate([-S.imag, S.real], 1).astype(BF16),          # [65, 256]
        "cDr": FH.real.astype(BF16),                                       # [128, 128]
        "cDi": (-FH.imag).astype(BF16),                                    # [128, 128]
        "cI": np.eye(128, dtype=np.float32).astype(BF16),                  # [128, 128]
    }
    return consts


def _build_program():
    from contextlib import ExitStack

    import concourse.bass as bass  # noqa: F401
    import concourse.mybir as mybir
    import concourse.tile as tile
    from concourse import bacc

    f32 = mybir.dt.float32
    bf = mybir.dt.bfloat16
    f8 = mybir.dt.float8e4
    DRow = mybir.MatmulPerfMode.DoubleRow
    AF = mybir.ActivationFunctionType
    ALU = mybir.AluOpType

    nc = bacc.Bacc("TRN2", target_bir_lowering=False, debug=False)

    xhw = nc.dram_tensor("xhw", [H, C, W], bf, kind="ExternalInput")
    w1pr = nc.dram_tensor("w1pr", [BS, 2, NBL, BS], f8, kind="ExternalInput")
    w1pi = nc.dram_tensor("w1pi", [BS, 2, NBL, BS], f8, kind="ExternalInput")
    w2r = nc.dram_tensor("w2r", [BS, NBL, BS], bf, kind="ExternalInput")
    w2i = nc.dram_tensor("w2i", [BS, NBL, BS], bf, kind="ExternalInput")
    nw2i = nc.dram_tensor("nw2i", [BS, NBL, BS], bf, kind="ExternalInput")
    w2Ad = nc.dram_tensor("w2Ad", [BS, NBL, BS], bf, kind="ExternalInput")
    w2Bd = nc.dram_tensor("w2Bd", [BS, NBL, BS], bf, kind="ExternalInput")
    c2d = nc.dram_tensor("c2d", [BS, NBL], f32, kind="ExternalInput")
    shp1d = nc.dram_tensor("shp1d", [BS, NBL], f32, kind="ExternalInput")
    shp1sd = nc.dram_tensor("shp1sd", [BS, NBL], f32, kind="ExternalInput")
    addrd = nc.dram_tensor("addrd", [BS, NBL], f32, kind="ExternalInput")
    addid = nc.dram_tensor("addid", [BS, NBL], f32, kind="ExternalInput")
    b2rd = nc.dram_tensor("b2rd", [BS, NBL], f32, kind="ExternalInput")
    b2id = nc.dram_tensor("b2id", [BS, NBL], f32, kind="ExternalInput")
    cF = nc.dram_tensor("cF", [H, 2 * H], bf, kind="ExternalInput")
    cBp = nc.dram_tensor("cBp", [W, 2, 2 * WF], f8, kind="ExternalInput")
    cE1 = nc.dram_tensor("cE1", [WF, 2 * W], bf, kind="ExternalInput")
    cE2 = nc.dram_tensor("cE2", [WF, 2 * W], bf, kind="ExternalInput")
    cDr = nc.dram_tensor("cDr", [H, H], bf, kind="ExternalInput")
    cDi = nc.dram_tensor("cDi", [H, H], bf, kind="ExternalInput")
    cI = nc.dram_tensor("cI", [128, 128], bf, kind="ExternalInput")
    outs = nc.dram_tensor("outs", [H, C, W], bf, kind="ExternalOutput")

    with ExitStack() as ctx:
        tc = ctx.enter_context(tile.TileContext(nc))
        consts = ctx.enter_context(tc.tile_pool(name="consts", bufs=1))
        xpool = ctx.enter_context(tc.tile_pool(name="xpool", bufs=2))
        zpool = ctx.enter_context(tc.tile_pool(name="zpool", bufs=1))
        planep = ctx.enter_context(tc.tile_pool(name="planep", bufs=1))
        pbufp = ctx.enter_context(tc.tile_pool(name="pbufp", bufs=3))
        mixp = ctx.enter_context(tc.tile_pool(name="mixp", bufs=2))
        outp = ctx.enter_context(tc.tile_pool(name="outp", bufs=5))
        psum = ctx.enter_context(tc.tile_pool(name="psum", bufs=2, space="PSUM"))

        # ---- stage-A DFT matrix first, then block-0 X prefetch (SP queue) ----
        cF_sb = consts.tile([H, 2 * H], bf)
        nc.sync.dma_start(cF_sb, cF[:])
        X16_first = xpool.tile([H, BS, W], bf, tag="xblk")
        for cc in range(0, BS, CSUB):
            nc.sync.dma_start(
                X16_first[:, cc: cc + CSUB, :], xhw[:, cc: cc + CSUB, :]
            )
        # ---- modulation vectors computed on host: shp1=shift+1, shp1_s=shp1/128,
        # addv = b1*shp1 + scale (ships 4 tiny f32 vectors instead of mwT) ----
        shp1 = consts.tile([BS, NBL], f32)
        shp1_s = consts.tile([BS, NBL], f32)
        addr_v = consts.tile([BS, NBL], f32)
        addi_v = consts.tile([BS, NBL], f32)
        nc.sync.dma_start(shp1, shp1d[:])
        nc.sync.dma_start(shp1_s, shp1sd[:])
        nc.sync.dma_start(addr_v, addrd[:])
        nc.sync.dma_start(addi_v, addid[:])

        cBp_sb = consts.tile([W, 2, 2 * WF], f8)
        nc.sync.dma_start(cBp_sb, cBp[:])
        cE1_sb = consts.tile([WF, 2 * W], bf)
        nc.sync.dma_start(cE1_sb, cE1[:])
        cE2_sb = consts.tile([WF, 2 * W], bf)
        nc.sync.dma_start(cE2_sb, cE2[:])
        cDr_sb = consts.tile([H, H], bf)
        nc.sync.dma_start(cDr_sb, cDr[:])
        cDi_sb = consts.tile([H, H], bf)
        nc.sync.dma_start(cDi_sb, cDi[:])
        cI_sb = consts.tile([128, 128], bf)
        nc.sync.dma_start(cI_sb, cI[:])

        # ---- block weights ----
        w1pr_sb = consts.tile([BS, 2, NBL, BS], f8)
        w1pi_sb = consts.tile([BS, 2, NBL, BS], f8)
        w2r_sb = consts.tile([BS, NBL, BS], bf)
        w2i_sb = consts.tile([BS, NBL, BS], bf)
        nw2i_sb = consts.tile([BS, NBL, BS], bf)
        w2A_sb = consts.tile([BS, NBL, BS], bf)   # w2r @ w2i (host)
        w2B_sb = consts.tile([BS, NBL, BS], bf)   # w2r - w2i @ w2i (host)
        for t_sb_, t_dr_ in ((w1pr_sb, w1pr), (w1pi_sb, w1pi),
                             (w2r_sb, w2r), (w2i_sb, w2i), (nw2i_sb, nw2i),
                             (w2A_sb, w2Ad), (w2B_sb, w2Bd)):
            nc.sync.dma_start(t_sb_, t_dr_[:])
        b2r_v = consts.tile([BS, NBL], f32)
        c2_v = consts.tile([BS, NBL], f32)        # b2r @ w2i + b2i (host)
        nc.sync.dma_start(b2r_v, b2rd[:])
        nc.sync.dma_start(c2_v, c2d[:])

        def stage_a(X16, Zbuf, deep=False):
            for cp in range(BS // 2):
                c = 2 * cp
                if deep and cp % 2 == 1:
                    # startup only: other rings are idle, deepen the pipeline
                    pA = psum.tile([128, 2, 2 * H], f32, tag="ps_m", bufs=3)
                else:
                    pA = psum.tile([128, 2, 2 * H], f32, tag="ps_b")
                nc.tensor.matmul(pA[:, 0, :], lhsT=X16[:, c, :], rhs=cF_sb,
                                 start=True, stop=True)
                nc.tensor.matmul(pA[:, 1, :], lhsT=X16[:, c + 1, :], rhs=cF_sb,
                                 start=True, stop=True)
                pAr = pA.rearrange("w c (r h) -> w r c h", r=2)
                if cp % 2 == 0:
                    nc.vector.tensor_copy(Zbuf[:, :, c: c + 2, :], pAr)
                else:
                    nc.scalar.copy(Zbuf[:, :, c: c + 2, :], pAr)

        # stage A of block 0 runs before the one-time setup sections so the
        # PE starts as soon as the first X chunk lands
        Zbuf_first = zpool.tile([W, 2, BS, H], f8, tag="zbuf")
        stage_a(X16_first, Zbuf_first, deep=True)



        # ---- main per-block pipeline ----
        for n in range(NBL):
            c0 = n * BS

            # resident X for this block: [h, c, w] bf16 (stage-A lhsT + residual)
            if n == 0:
                X16 = X16_first
            else:
                X16 = xpool.tile([H, BS, W], bf, tag="xblk")
                for cc in range(0, BS, CSUB):
                    nc.sync.dma_start(
                        X16[:, cc: cc + CSUB, :],
                        xhw[:, c0 + cc: c0 + cc + CSUB, :],
                    )

            # ---- stage A: Z^T = X_c^T @ [Fr|Fi] -> Zbuf [w, c, h'Zr|h'Zi] ----
            if n == 0:
                Zbuf = Zbuf_first
            else:
                Zbuf = zpool.tile([W, 2, BS, H], f8, tag="zbuf")
                stage_a(X16, Zbuf)

            # ---- fused B -> mix -> T per chunk of HC h' rows ----
            # merged planes: Wpl[:, 0] = real, Wpl[:, 1] = imag
            Wpl = planep.tile([WF, 2, H, BS], bf, tag="wpl")
            for ch_i in range(H // HC):
                h0 = ch_i * HC
                arch = mixp.tile([BS, 2, HC, WF], f8, tag="arch", bufs=5)
                for j2 in range(HC // 2):
                    pB = psum.tile([BS, 2, 2 * WF], f32, tag="ps_b")
                    for j in range(2):
                        hj = h0 + j2 * 2 + j
                        nc.tensor.matmul(
                            pB[:, j, :], lhsT=Zbuf[:, :, :, hj], rhs=cBp_sb,
                            start=True, stop=True, perf_mode=DRow,
                        )
                    pBr = pB.rearrange("p j (r f) -> p r j f", r=2)
                    if j2 == 0:
                        nc.scalar.copy(arch[:, :, 0:2, :], pBr)
                    else:
                        nc.vector.tensor_copy(arch[:, :, 2:4, :], pBr)
                # layer 1 (DoubleRow: k-tiles = (Ar, Ai))
                p1r = psum.tile([BS, HC, WF], f32, tag="ps_m", bufs=3)
                nc.tensor.matmul(p1r, lhsT=w1pr_sb[:, :, n, :], rhs=arch,
                                 start=True, stop=True, perf_mode=DRow)
                p1i = psum.tile([BS, HC, WF], f32, tag="ps_m", bufs=3)
                nc.tensor.matmul(p1i, lhsT=w1pi_sb[:, :, n, :], rhs=arch,
                                 start=True, stop=True, perf_mode=DRow)
                r1 = mixp.tile([BS, HC, WF], bf, tag="r1", bufs=5)
                i1 = mixp.tile([BS, HC, WF], bf, tag="i1", bufs=4)
                nc.scalar.activation(
                    r1, p1r, AF.Relu, bias=addr_v[:, n: n + 1],
                    scale=shp1_s[:, n: n + 1]
                )
                nc.scalar.activation(
                    i1, p1i, AF.Relu, bias=addi_v[:, n: n + 1],
                    scale=shp1_s[:, n: n + 1]
                )
                # layer 2: r2 = w2r@r1 - w2i@i1 + b2r ; i2 = A2@r1 + B2@i1 + c2
                p2r = psum.tile([BS, HC, WF], f32, tag="ps_m", bufs=3)
                nc.tensor.matmul(p2r, lhsT=w2r_sb[:, n, :], rhs=r1, start=True, stop=False)
                nc.tensor.matmul(p2r, lhsT=nw2i_sb[:, n, :], rhs=i1, start=False, stop=True)
                p2i = psum.tile([BS, HC, WF], f32, tag="ps_m", bufs=3)
                nc.tensor.matmul(p2i, lhsT=w2A_sb[:, n, :], rhs=r1, start=True, stop=False)
                nc.tensor.matmul(p2i, lhsT=w2B_sb[:, n, :], rhs=i1, start=False, stop=True)
                # biased r2|i2 staged in one tile: rb2[:, 0] = r2, rb2[:, 1] = i2
                rb2 = mixp.tile([BS, 2, HC, WF], bf, tag="rb2", bufs=4)
                nc.scalar.activation(rb2[:, 0, :, :], p2r, AF.Identity,
                                     bias=b2r_v[:, n: n + 1])
                nc.vector.tensor_scalar(rb2[:, 1, :, :], p2i, c2_v[:, n: n + 1],
                                        None, ALU.add)
                # softshrink(v) = v - clip(v, -lam, lam): clip on Pool, sub on DVE
                sab = mixp.tile([BS, 2, HC, WF], bf, tag="sab", bufs=4)
                nc.gpsimd.tensor_scalar(sab, rb2, -LAM, LAM, ALU.max, ALU.min)
                R2I2 = mixp.tile([BS, 2, HC, WF], bf, tag="R2I2", bufs=4)
                nc.vector.tensor_sub(R2I2[:, 0, :, :], rb2[:, 0, :, :],
                                     sab[:, 0, :, :])
                nc.gpsimd.tensor_sub(R2I2[:, 1, :, :], rb2[:, 1, :, :],
                                     sab[:, 1, :, :])
                # T: pivot [c, wf] -> [wf, c]; one merged psum bank, one evict
                pT = psum.tile([WF, 2, HC, BS], bf, tag="ps_t", bufs=1)
                for j in range(HC):
                    nc.tensor.transpose(pT[:, 0, j, :], R2I2[:, 0, j, :],
                                        cI_sb[0:BS, 0:BS])
                    nc.tensor.transpose(pT[:, 1, j, :], R2I2[:, 1, j, :],
                                        cI_sb[0:BS, 0:BS])
                nc.vector.tensor_copy(Wpl[:, :, h0: h0 + HC, :], pT)

            # ---- stages E' + D' in sub-groups of CSUB channels ----
            for sub in range(BS // CSUB):
                cb = sub * CSUB
                Pbuf = pbufp.tile([H, CSUB, 2 * H], bf, tag="pbuf")
                for cp in range(CSUB // 2):
                    c = cb + 2 * cp
                    if n == NBL - 1:
                        # drain: mix + A/B rings are idle after the last mix
                        if cp % 2 == 0:
                            pE = psum.tile([128, 2, 2 * H], f32, tag="ps_m", bufs=3)
                        else:
                            pE = psum.tile([128, 2, 2 * H], f32, tag="ps_b")
                    else:
                        pE = psum.tile([128, 2, 2 * H], f32, tag="ps_a")
                    for q in range(2):
                        nc.tensor.matmul(
                            pE[:, q, :], lhsT=Wpl[:, 0, :, c + q], rhs=cE1_sb,
                            start=True, stop=False,
                        )
                        nc.tensor.matmul(
                            pE[:, q, :], lhsT=Wpl[:, 1, :, c + q], rhs=cE2_sb,
                            start=False, stop=True,
                        )
                    if cp % 2 == 0:
                        nc.vector.tensor_copy(Pbuf[:, 2 * cp: 2 * cp + 2, :], pE)
                    else:
                        nc.scalar.copy(Pbuf[:, 2 * cp: 2 * cp + 2, :], pE)
                # D': out = FHr@Pr - FHi@Pi + x
                for g in range(CSUB // 4):
                    gc = 4 * g
                    pD = psum.tile([H, 4, W], f32, tag="ps_a")
                    nc.tensor.matmul(
                        pD, lhsT=cDr_sb, rhs=Pbuf[:, gc: gc + 4, 0:H],
                        start=True, stop=False,
                    )
                    ot = outp.tile([H, 4, W], bf, tag="ot")
                    if g % 3 != 2:
                        # residual folded into the DVE eviction (same cost as copy)
                        nc.tensor.matmul(
                            pD, lhsT=cDi_sb, rhs=Pbuf[:, gc: gc + 4, H: 2 * H],
                            start=False, stop=True,
                        )
                        nc.vector.tensor_add(ot, pD, X16[:, cb + gc: cb + gc + 4, :])
                    else:
                        # ACT can't tensor+tensor: accumulate x via identity matmul
                        nc.tensor.matmul(
                            pD, lhsT=cDi_sb, rhs=Pbuf[:, gc: gc + 4, H: 2 * H],
                            start=False, stop=False,
                        )
                        nc.tensor.matmul(
                            pD, lhsT=cI_sb, rhs=X16[:, cb + gc: cb + gc + 4, :],
                            start=False, stop=True,
                        )
                        nc.scalar.copy(ot, pD)
                    nc.sync.dma_start(
                        outs[:, c0 + cb + gc: c0 + cb + gc + 4, :], ot
                    )

    nc.compile()
    return nc


_CACHE = {}


def _get_program():
    if "nc" not in _CACHE:
        _CACHE["nc"] = _build_program()
    return _CACHE["nc"]


def kernel(**inputs):
    x = np.asarray(inputs["x"], dtype=np.float32)
    t = np.asarray(inputs["t"], dtype=np.float32)
    w1 = np.asarray(inputs["w1"], dtype=np.float32)
    b1 = np.asarray(inputs["b1"], dtype=np.float32)
    w2 = np.asarray(inputs["w2"], dtype=np.float32)
    b2 = np.asarray(inputs["b2"], dtype=np.float32)
    mod_w = np.asarray(inputs["mod_w"], dtype=np.float32)
    mod_b = np.asarray(inputs["mod_b"], dtype=np.float32)

    from concourse.bass_utils import run_bass_kernel_spmd

    nc = _get_program()
    consts = _host_consts()

    silu_t = t / (1.0 + np.exp(-t))
    mod_full = silu_t @ mod_w.T + mod_b            # (B, 2*DIM)
    mod_full = mod_full.reshape(B_FULL, NB, 2 * BS)
    in_maps = []
    for core in range(N_CORES):
        b = core // 2
        sh = mod_full[b, :, :BS] + 1.0             # (NB, BS)
        sc = mod_full[b, :, BS:]
        n0 = (core % 2) * NBL
        cs = slice(n0 * BS, n0 * BS + C)
        rs = slice(n0 * 2 * BS, (n0 + NBL) * 2 * BS)
        W1 = w1[:, n0: n0 + NBL]          # [2, NBL, BS, BS] (d, k)
        W2 = w2[:, n0: n0 + NBL]
        im = {
            "xhw": np.ascontiguousarray(
                x[b, cs].transpose(1, 0, 2)).astype(BF16),          # [h, c, w]
            "w1pr": np.ascontiguousarray(np.stack(
                [8.0 * W1[0], -8.0 * W1[1]], 0).transpose(2, 0, 1, 3)).astype(F8),
            "w1pi": np.ascontiguousarray(np.stack(
                [8.0 * W1[1], 8.0 * W1[0]], 0).transpose(2, 0, 1, 3)).astype(F8),
            "w2r": np.ascontiguousarray(W2[0].transpose(1, 0, 2)).astype(BF16),
            "w2i": np.ascontiguousarray(W2[1].transpose(1, 0, 2)).astype(BF16),
            "nw2i": np.ascontiguousarray(-W2[1].transpose(1, 0, 2)).astype(BF16),
            "w2Ad": np.ascontiguousarray(
                np.einsum("nde,nek->dnk", W2[0], W2[1])).astype(BF16),
            "w2Bd": np.ascontiguousarray(
                (W2[0] - np.einsum("nde,nek->ndk", W2[1], W2[1])
                 ).transpose(1, 0, 2)).astype(BF16),
            "c2d": np.ascontiguousarray(
                (np.einsum("nd,ndk->nk", b2[0, n0: n0 + NBL], W2[1])
                 + b2[1, n0: n0 + NBL]).T).astype(np.float32),
            "shp1d": np.ascontiguousarray(sh[n0: n0 + NBL].T),
            "shp1sd": np.ascontiguousarray(sh[n0: n0 + NBL].T / 128.0),
            "addrd": np.ascontiguousarray(
                (b1[0, n0: n0 + NBL] * sh[n0: n0 + NBL]
                 + sc[n0: n0 + NBL]).T),
            "addid": np.ascontiguousarray(
                (b1[1, n0: n0 + NBL] * sh[n0: n0 + NBL]
                 + sc[n0: n0 + NBL]).T),
            "b2rd": np.ascontiguousarray(b2[0, n0: n0 + NBL].T),
            "b2id": np.ascontiguousarray(b2[1, n0: n0 + NBL].T),
        }
        im.update(consts)
        in_maps.append(im)

    res = run_bass_kernel_spmd(
        nc, in_maps, core_ids=list(range(N_CORES))
    )

    out = np.empty((B_FULL, DIM, H, W), dtype=np.float32)
    for core in range(N_CORES):
        b = core // 2
        n0 = (core % 2) * NBL
        cs = slice(n0 * BS, n0 * BS + C)
        out[b, cs] = res.results[core]["outs"].astype(np.float32).transpose(1, 0, 2)
    return out

